# revision 26
# baseline (speedup 1.0000x reference)
"""Trainium2 Bass kernel for nn_LinearTemporalCrossAttention (v2).

Feature-major main pipeline; bf16 IO; single ACT table set (exp/tanh/copy);
LN via bn_stats + quake rsqrt on DVE; block-diagonal attention tiles.

Reference computation (per batch b):
  xn = LN(x) ; tn = LN(xf)
  q = softmax((xn @ Wq.T + bq) per-head, axis=-1)        # [T, H, Dh]
  k = softmax((tn @ Wk.T + bk) per-head, axis=N)         # [N, H, Dh]
  v = tn @ Wv.T + bv                                     # [N, H, Dh]
  attn = einsum('nhd,nhl->hdl', k, v)
  y = einsum('thd,hdl->thl', q, attn)                    # [T, D]
  emb_out = silu(emb) @ emb_w.T + emb_b ; scale, shift = split(emb_out)
  h = LN(y) * (1+scale) + shift
  h = silu(h) @ out_w.T + out_b
  out = x + sigmoid(gate) * h

Sharding: data-parallel over batch B=64 across 8 cores (8 batches/core).
"""

import os
import sys
import numpy as np

for _p in ("/opt/trn_rl_repo", "/root/.axon_site/_ro/trn_rl_repo"):
    if _p not in sys.path:
        sys.path.insert(0, _p)

import concourse.bass as bass
import concourse.bacc as bacc
import concourse.tile as tile
import concourse.mybir as mybir
from concourse import bass_utils

F32 = mybir.dt.float32
BF16 = mybir.dt.bfloat16
F8 = mybir.dt.float8e4
I32 = mybir.dt.int32
W8SCALE = 64.0
AX = mybir.AxisListType
OP = mybir.AluOpType
AF = mybir.ActivationFunctionType

B, T, N, D, H, DT, DE = 64, 1024, 77, 512, 8, 256, 2048
Dh = D // H  # 64
NCORES = 8
B_LOC = B // NCORES  # 8
ST = 512
N_ST = T // ST  # 2
PT = 128
NTT = ST // PT  # 4
NKT = D // PT  # 4
NKT_T = DT // PT  # 2
NKT_E = DE // PT  # 16
QUAKE_C = 0x5F3759DF


def _quake_rsqrt(nc, pool, shape, veps, tag, iters=2):
    """rstd = veps**-0.5 via quake + Newton iters, all on DVE (f32)."""
    y = pool.tile(shape, F32, tag=tag + "_y")
    ih = pool.tile(shape, I32, tag=tag + "_i")
    nc.vector.tensor_scalar(ih[:], veps[:].bitcast(I32), 1, None,
                            op0=OP.arith_shift_right)
    nc.vector.tensor_scalar(y[:].bitcast(I32), ih[:], -1, QUAKE_C,
                            op0=OP.mult, op1=OP.add)
    sq = pool.tile(shape, F32, tag=tag + "_s")
    hh = pool.tile(shape, F32, tag=tag + "_h")
    for _ in range(iters):
        nc.vector.tensor_tensor(sq[:], y[:], y[:], op=OP.mult)
        nc.vector.tensor_tensor(sq[:], sq[:], veps[:], op=OP.mult)
        nc.vector.tensor_scalar(hh[:], sq[:], -0.5, 1.5, op0=OP.mult,
                                op1=OP.add)
        nc.vector.tensor_tensor(y[:], y[:], hh[:], op=OP.mult)
    return y


def build_program(B_loc):
    nc = bacc.Bacc("TRN2", target_bir_lowering=False, debug=False,
                   enable_asserts=False)

    x_d = nc.dram_tensor("x", [B_loc, T, D], BF16, kind="ExternalInput").ap()
    xf_d = nc.dram_tensor("xf", [B_loc, N, DT], BF16, kind="ExternalInput").ap()
    emb_d = nc.dram_tensor("emb", [B_loc, DE], BF16, kind="ExternalInput").ap()
    wq_d = nc.dram_tensor("wq", [D, D], F8, kind="ExternalInput").ap()
    wk_d = nc.dram_tensor("wk", [DT, D], BF16, kind="ExternalInput").ap()
    wv_d = nc.dram_tensor("wv", [DT, D], BF16, kind="ExternalInput").ap()
    ow_d = nc.dram_tensor("ow", [D, D], F8, kind="ExternalInput").ap()
    wemb_d = nc.dram_tensor("wemb", [DE, 2 * D], BF16, kind="ExternalInput").ap()
    bq_d = nc.dram_tensor("bq", [PT, NKT], F32, kind="ExternalInput").ap()
    bk_d = nc.dram_tensor("bk", [1, D], BF16, kind="ExternalInput").ap()
    bv_d = nc.dram_tensor("bv", [1, D], BF16, kind="ExternalInput").ap()
    lnsw_d = nc.dram_tensor("lnsw", [1, D], F32, kind="ExternalInput").ap()
    lnsb_d = nc.dram_tensor("lnsb", [1, D], F32, kind="ExternalInput").ap()
    mask8_d = nc.dram_tensor("mask8", [PT, NKT, H], BF16,
                             kind="ExternalInput").ap()
    selq_d = nc.dram_tensor("selq", [H, NKT, PT], BF16,
                            kind="ExternalInput").ap()
    ident_d = nc.dram_tensor("ident", [PT, PT], BF16, kind="ExternalInput").ap()
    identg_d = nc.dram_tensor("identg", [PT, PT], BF16, kind="ExternalInput").ap()
    ident32_d = nc.dram_tensor("ident32", [PT, PT], F32,
                               kind="ExternalInput").ap()
    out_d = nc.dram_tensor("out", [B_loc, T, D], BF16, kind="ExternalOutput").ap()
    dbg = {}
    if os.environ.get("KDBG") == "1":
        dbg["effSH"] = nc.dram_tensor("d_effSH", [PT, NKT, 2 * B_loc], F32,
                                      kind="ExternalOutput").ap()
        dbg["attn_bd"] = nc.dram_tensor("d_attn", [PT, B_loc * NKT, PT], BF16,
                                        kind="ExternalOutput").ap()
        dbg["expq"] = nc.dram_tensor("d_expq", [PT, NKT, ST], BF16,
                                     kind="ExternalOutput").ap()
        dbg["qrec"] = nc.dram_tensor("d_qrec", [H, ST], BF16,
                                     kind="ExternalOutput").ap()
        dbg["ybf"] = nc.dram_tensor("d_ybf", [PT, NKT, ST], BF16,
                                    kind="ExternalOutput").ap()
        dbg["qn"] = nc.dram_tensor("d_qn", [PT, NKT, ST], BF16,
                                   kind="ExternalOutput").ap()
        dbg["abrow"] = nc.dram_tensor("d_abrow", [1, 2 * ST], BF16,
                                      kind="ExternalOutput").ap()
        dbg["hs"] = nc.dram_tensor("d_hs", [PT, NKT, ST], BF16,
                                   kind="ExternalOutput").ap()
        dbg["xsf"] = nc.dram_tensor("d_xsf", [PT, NKT, ST], BF16,
                                    kind="ExternalOutput").ap()

    with tile.TileContext(nc) as tc:
        _emit(tc, nc, B_loc, x_d, xf_d, emb_d, wq_d, wk_d, wv_d, ow_d, wemb_d,
              bq_d, bk_d, bv_d, lnsw_d, lnsb_d, mask8_d, selq_d, ident_d,
              identg_d, ident32_d, out_d, dbg)

    nc.compile()
    return nc


def _emit(tc, nc, B_loc, x_d, xf_d, emb_d, wq_d, wk_d, wv_d, ow_d, wemb_d,
          bq_d, bk_d, bv_d, lnsw_d, lnsb_d, mask8_d, selq_d, ident_d,
          identg_d, ident32_d, out_d, dbg=None):
    dbg = dbg or {}
    from contextlib import ExitStack
    ctx = ExitStack()
    with ctx:
        wpool = ctx.enter_context(tc.tile_pool(name="weights", bufs=1))
        persist = ctx.enter_context(tc.tile_pool(name="persist", bufs=1))

        # ---- persistent weights ----
        wq = wpool.tile([PT, NKT, D], F8, tag="wq")
        nc.sync.dma_start(wq[:], wq_d.rearrange("(k p) d -> p k d", p=PT))
        ow = wpool.tile([PT, NKT, D], F8, tag="ow")
        nc.sync.dma_start(ow[:], ow_d.rearrange("(k p) d -> p k d", p=PT))
        wk = wpool.tile([PT, NKT_T, D], BF16, tag="wk")
        nc.sync.dma_start(wk[:], wk_d.rearrange("(k p) d -> p k d", p=PT))
        wv = wpool.tile([PT, NKT_T, D], BF16, tag="wv")
        nc.sync.dma_start(wv[:], wv_d.rearrange("(k p) d -> p k d", p=PT))
        bq = wpool.tile([PT, NKT], F32, tag="bq")
        nc.sync.dma_start(bq[:], bq_d)
        bk = wpool.tile([1, D], BF16, tag="bk")
        nc.sync.dma_start(bk[:], bk_d)
        bv = wpool.tile([1, D], BF16, tag="bv")
        nc.sync.dma_start(bv[:], bv_d)
        mask8 = wpool.tile([PT, NKT, H], BF16, tag="mask8")
        nc.sync.dma_start(mask8[:], mask8_d)
        selq = wpool.tile([H, NKT, PT], BF16, tag="selq")
        nc.sync.dma_start(selq[:], selq_d)
        ones1r = wpool.tile([1, PT], BF16, tag="ones1r")
        nc.vector.memset(ones1r[:], 1.0)
        ident = wpool.tile([PT, PT], BF16, tag="ident")
        nc.sync.dma_start(ident[:], ident_d)
        identg = wpool.tile([PT, PT], BF16, tag="identg")
        nc.sync.dma_start(identg[:], identg_d)
        ident32 = wpool.tile([PT, PT], F32, tag="ident32")
        nc.sync.dma_start(ident32[:], ident32_d)
        lnsw = wpool.tile([1, D], F32, tag="lnsw")
        nc.sync.dma_start(lnsw[:], lnsw_d)
        lnsb = wpool.tile([1, D], F32, tag="lnsb")
        nc.sync.dma_start(lnsb[:], lnsb_d)
        ones77 = wpool.tile([N, 1], BF16, tag="ones77")
        nc.vector.memset(ones77[:], 1.0)
        ones1x = wpool.tile([1, N], BF16, tag="ones1x")
        nc.vector.memset(ones1x[:], 1.0)
        ones128 = wpool.tile([PT, 1], BF16, tag="ones128")
        nc.vector.memset(ones128[:], 1.0)

        # persistent per-batch products
        attn_bd = persist.tile([PT, B_loc * NKT, PT], BF16, tag="attn_bd")
        nc.vector.memset(attn_bd[:], 0.0)
        effSH = persist.tile([PT, NKT, 2 * B_loc], F32, tag="effSH")

        # ================= Phase E: emb -> effS/effH (FM) ================
        with tc.tile_pool(name="embp", bufs=1) as embp, \
             tc.tile_pool(name="embps", bufs=1, space="PSUM") as embps:
            wemb = embp.tile([PT, NKT_E, 2 * D], BF16, tag="wemb")
            nc.sync.dma_start(wemb[:],
                              wemb_d.rearrange("(k p) d -> p k d", p=PT))
            embt = embp.tile([B_loc, DE], BF16, tag="embt")
            nc.sync.dma_start(embt[:], emb_d)
            # silu(e)*2 = e*(1+tanh(e/2)); the 0.5 is folded into wemb
            the = embp.tile([B_loc, DE], BF16, tag="the")
            nc.scalar.activation(the[:], embt[:], AF.Tanh, bias=0.0, scale=0.5)
            esi = embp.tile([B_loc, DE], BF16, tag="esi")
            nc.vector.scalar_tensor_tensor(esi[:], the[:], 1.0, embt[:],
                                           op0=OP.add, op1=OP.mult)
            esiT_ps = embps.tile([PT, NKT_E, B_loc], BF16, tag="etp")
            for kt in range(NKT_E):
                nc.tensor.transpose(esiT_ps[:, kt, :],
                                    esi[:, kt * PT:(kt + 1) * PT],
                                    ident[:B_loc, :B_loc])
            esiT = embp.tile([PT, NKT_E, B_loc], BF16, tag="esiT")
            nc.vector.tensor_copy(esiT[:], esiT_ps[:])
            emb_ps = embps.tile([B_loc, 2, D], F32, tag="embmm")
            for half in range(2):
                for kt in range(NKT_E):
                    nc.tensor.matmul(emb_ps[:, half, :], esiT[:, kt, :],
                                     wemb[:, kt, half * D:(half + 1) * D],
                                     start=(kt == 0), stop=(kt == NKT_E - 1))
            # effS = (1+scale)*lnsw ; effH = (1+scale)*lnsb + shift
            lnswb = embp.tile([1, D], BF16, tag="lnswb")
            nc.vector.tensor_copy(lnswb[:], lnsw[:])
            lnsbb = embp.tile([1, D], BF16, tag="lnsbb")
            nc.vector.tensor_copy(lnsbb[:], lnsb[:])
            ln8_ps = embps.tile([B_loc, 2, D], F32, tag="ln8")
            nc.tensor.matmul(ln8_ps[:, 0, :], ones1r[:, :B_loc], lnswb[:],
                             start=True, stop=True)
            nc.tensor.matmul(ln8_ps[:, 1, :], ones1r[:, :B_loc], lnsbb[:],
                             start=True, stop=True)
            ln8 = embp.tile([B_loc, 2, D], F32, tag="ln8sb")
            nc.vector.tensor_copy(ln8[:], ln8_ps[:])
            effS8 = embp.tile([B_loc, D], BF16, tag="effS8")
            nc.vector.scalar_tensor_tensor(effS8[:], emb_ps[:, 0, :], 1.0,
                                           ln8[:, 0, :], op0=OP.add,
                                           op1=OP.mult)
            t8 = embp.tile([B_loc, D], F32, tag="t8")
            nc.vector.scalar_tensor_tensor(t8[:], emb_ps[:, 0, :], 1.0,
                                           ln8[:, 1, :], op0=OP.add,
                                           op1=OP.mult)
            effH8 = embp.tile([B_loc, D], BF16, tag="effH8")
            nc.vector.tensor_add(effH8[:], t8[:], emb_ps[:, 1, :])
            # transpose to FM: effSH[p, kt, b] / [p, kt, B_loc+b]
            eff_ps = embps.tile([PT, NKT, 2 * B_loc], BF16, tag="effps")
            for kt in range(NKT):
                nc.tensor.transpose(eff_ps[:, kt, 0:B_loc],
                                    effS8[:, kt * PT:(kt + 1) * PT],
                                    ident[:B_loc, :B_loc])
                nc.tensor.transpose(eff_ps[:, kt, B_loc:2 * B_loc],
                                    effH8[:, kt * PT:(kt + 1) * PT],
                                    ident[:B_loc, :B_loc])
            nc.vector.tensor_copy(effSH[:], eff_ps[:])
            if "effSH" in dbg:
                nc.sync.dma_start(dbg["effSH"], effSH[:])

        io = ctx.enter_context(tc.tile_pool(name="io", bufs=3))
        work = ctx.enter_context(tc.tile_pool(name="work", bufs=2))
        stat = ctx.enter_context(tc.tile_pool(name="stat", bufs=3))
        ps_tp = ctx.enter_context(
            tc.tile_pool(name="ps_tp", bufs=1, space="PSUM"))
        ps_mm = ctx.enter_context(
            tc.tile_pool(name="ps_mm", bufs=4, space="PSUM"))
        ps_misc = ctx.enter_context(
            tc.tile_pool(name="ps_misc", bufs=2, space="PSUM"))

        # ================= Phase K: xf -> attn_bd (per batch) ============
        kp = ctx.enter_context(tc.tile_pool(name="kp", bufs=2))
        kstat = ctx.enter_context(tc.tile_pool(name="kstat", bufs=2))

        def phase_k(b):
            xft = kp.tile([N, DT], BF16, tag="xft")
            nc.sync.dma_start(xft[:], xf_d[b])
            bns = kstat.tile([N, 6], F32, tag="bns")
            nc.vector.bn_stats(bns[:], xft[:])
            mv = kstat.tile([N, 2], F32, tag="mv")
            nc.vector.bn_aggr(mv[:], bns[:])
            veps = kstat.tile([N, 1], F32, tag="veps")
            nc.vector.tensor_scalar(veps[:], mv[:, 1:2], 1.0, 1e-5,
                                    op0=OP.mult, op1=OP.add)
            rstd = _quake_rsqrt(nc, kstat, [N, 1], veps, "kq")
            nmr = kstat.tile([N, 1], F32, tag="nmr")
            nc.vector.scalar_tensor_tensor(nmr[:], mv[:, 0:1], -1.0,
                                           rstd[:], op0=OP.mult, op1=OP.mult)
            xfs = kp.tile([N, DT], BF16, tag="xfs")
            nc.vector.tensor_scalar(xfs[:], xft[:], rstd[:], nmr[:],
                                    op0=OP.mult, op1=OP.add)
            xfsT_ps = ps_misc.tile([PT, NKT_T, 80], BF16, tag="misc")
            for kt in range(NKT_T):
                nc.tensor.transpose(xfsT_ps[:, kt, 0:N],
                                    xfs[:, kt * PT:(kt + 1) * PT],
                                    ident[:N, :N])
            xfsT = kp.tile([PT, NKT_T, N], BF16, tag="xfsT")
            nc.vector.tensor_copy(xfsT[:], xfsT_ps[:, :, 0:N])
            kvsb = kp.tile([N, 2, D], BF16, tag="kvsb")
            for i, (w, br) in enumerate(((wk, bk), (wv, bv))):
                kv_ps = ps_mm.tile([PT, ST], F32, tag="mm")
                for kt in range(NKT_T):
                    nc.tensor.matmul(kv_ps[:N, :], xfsT[:, kt, :],
                                     w[:, kt, :], start=(kt == 0),
                                     stop=False)
                nc.tensor.matmul(kv_ps[:N, :], ones1x[:, :N], br[:],
                                 start=False, stop=True)
                if i == 0:
                    nc.scalar.activation(kvsb[:, 0, :], kv_ps[:N, :], AF.Exp)
                else:
                    nc.vector.tensor_copy(kvsb[:, 1, :], kv_ps[:N, :])
            expk = kvsb[:, 0, :]
            vsb = kvsb[:, 1, :]
            srow_ps = ps_misc.tile([1, D], F32, tag="misc")
            nc.tensor.matmul(srow_ps[:], ones77[:], expk, start=True,
                             stop=True)
            srow = kstat.tile([1, D], F32, tag="srowsb")
            nc.vector.tensor_copy(srow[:], srow_ps[:])
            sT_ps = ps_misc.tile([PT, NKT, 4], F32, tag="misc")
            for ft in range(NKT):
                nc.tensor.transpose(sT_ps[:, ft, 0:1],
                                    srow[:, ft * PT:(ft + 1) * PT],
                                    ident32[:1, :1])
            srecT = kstat.tile([PT, NKT], F32, tag="srecT")
            nc.vector.reciprocal_approx_fast(srecT[:], sT_ps[:, :, 0])
            for kt in range(NKT):
                attn_ps = ps_mm.tile([PT, ST], F32, tag="mm")
                for j in range(2):
                    h = 2 * kt + j
                    po = j * Dh
                    nc.tensor.matmul(attn_ps[po:po + Dh, po:po + Dh],
                                     expk[:, h * Dh:(h + 1) * Dh],
                                     vsb[:, h * Dh:(h + 1) * Dh],
                                     start=True, stop=True)
                for j in range(2):
                    po = j * Dh
                    nc.vector.tensor_scalar(
                        attn_bd[po:po + Dh, b * NKT + kt, po:po + Dh],
                        attn_ps[po:po + Dh, po:po + Dh],
                        srecT[po:po + Dh, kt:kt + 1], None, op0=OP.mult)

        # ================= Phase M: 2-stage SW-pipelined main loop =======
        if "attn_bd" in dbg:
            nc.sync.dma_start(dbg["attn_bd"], attn_bd[:])

        def front_a(T):
            b, st, dump = T["b"], T["st"], T["dump"]
            tok0 = st * ST
            x_in = io.tile([PT, NTT, D], BF16, tag="x_in")
            nc.sync.dma_start(
                x_in[:], x_d[b, tok0:tok0 + ST, :].rearrange(
                    "(tt p) d -> p tt d", p=PT))
            T["x_in"] = x_in
            # ---- LN1 (TM) ----
            bns = stat.tile([PT, NTT, 6], F32, tag="bns")
            for t in range(NTT):
                nc.vector.bn_stats(bns[:, t, :], x_in[:, t, :])
            mv = stat.tile([PT, NTT, 2], F32, tag="mv")
            for t in range(NTT):
                nc.vector.bn_aggr(mv[:, t, :], bns[:, t, :])
            rstd = _quake_rsqrt(nc, stat, [PT, NTT], mv[:, :, 1], "mq",
                                 iters=1)
            nmr = stat.tile([PT, NTT], F32, tag="nmr")
            nc.vector.scalar_tensor_tensor(nmr[:], mv[:, :, 0], -1.0,
                                           rstd[:], op0=OP.mult, op1=OP.mult)
            xs = work.tile([PT, NTT, D], BF16, tag="xs")
            for t in range(NTT):
                nc.vector.tensor_scalar(xs[:, t, :], x_in[:, t, :],
                                        rstd[:, t:t + 1], nmr[:, t:t + 1],
                                        op0=OP.mult, op1=OP.add)
            # ---- T1: transpose to FM ----
            xsT_ps = ps_tp.tile([PT, NKT, ST], BF16, tag="tp")
            for kt in range(NKT):
                for t in range(NTT):
                    nc.tensor.transpose(
                        xsT_ps[:, kt, t * PT:(t + 1) * PT],
                        xs[:, t, kt * PT:(kt + 1) * PT], ident[:])
            xsf = work.tile([PT, NKT, ST], F8, tag="xsf")
            nc.vector.tensor_copy(xsf[:, 0:2, :], xsT_ps[:, 0:2, :])
            nc.vector.tensor_copy(xsf[:, 2:4, :], xsT_ps[:, 2:4, :])
            # ---- q-proj (fp8 DoubleRow, FM out) + exp ----
            expq = work.tile([PT, NKT, ST], BF16, tag="expq")
            for fo in range(NKT):
                q_ps = ps_mm.tile([PT, ST], F32, tag="mm")
                for k2 in range(2):
                    nc.tensor.matmul(
                        q_ps[:],
                        wq[:, 2 * k2:2 * k2 + 2, fo * PT:(fo + 1) * PT],
                        xsf[:, 2 * k2:2 * k2 + 2, :], start=(k2 == 0),
                        stop=(k2 == 1),
                        perf_mode=mybir.MatmulPerfMode.DoubleRow)
                nc.scalar.activation(expq[:, fo, :], q_ps[:], AF.Exp,
                                     bias=bq[:, fo:fo + 1],
                                     scale=1.0 / W8SCALE)
            T["expq"] = expq

        def front_b(T):
            expq = T["expq"]
            # ---- softmax sums (per head) ----
            s_ps = ps_misc.tile([H, ST], F32, tag="misc")
            for kt in range(NKT):
                nc.tensor.matmul(s_ps[:], mask8[:, kt, :], expq[:, kt, :],
                                 start=(kt == 0), stop=(kt == NKT - 1))
            qrec = stat.tile([H, ST], F32, tag="qrec")
            nc.vector.reciprocal_approx_fast(qrec[:], s_ps[:])
            qrec16 = stat.tile([H, ST], BF16, tag="qrec16")
            nc.vector.tensor_copy(qrec16[:], qrec[:])
            qrecB = work.tile([PT, NKT, ST], BF16, tag="qrecB")
            for kt in range(NKT):
                qb_ps = ps_mm.tile([PT, ST], F32, tag="mm")
                nc.tensor.matmul(qb_ps[:], selq[:, kt, :], qrec16[:],
                                 start=True, stop=True)
                nc.scalar.copy(qrecB[:, kt, :], qb_ps[:])
            T["qrecB"] = qrecB
            if T["dump"]:
                nc.sync.dma_start(dbg["expq"], expq[:])
                nc.sync.dma_start(dbg["qrec"], qrec16[:])

        def front_c(T):
            b, expq, qrecB = T["b"], T["expq"], T["qrecB"]
            qn = work.tile([PT, NKT, ST], BF16, tag="qn")
            nc.vector.tensor_tensor(qn[:, 0:2, :], expq[:, 0:2, :],
                                    qrecB[:, 0:2, :], op=OP.mult)
            nc.vector.tensor_tensor(qn[:, 2:4, :], expq[:, 2:4, :],
                                    qrecB[:, 2:4, :], op=OP.mult)
            # ---- y einsum (block-diag attn) ----
            ybf = io.tile([PT, NKT, ST], BF16, tag="ybf")
            for kt in range(NKT):
                y_ps = ps_mm.tile([PT, ST], F32, tag="mm")
                nc.tensor.matmul(y_ps[:], attn_bd[:, b * NKT + kt, :],
                                 qn[:, kt, :], start=True, stop=True)
                if kt % 2 == 0:
                    nc.vector.tensor_copy(ybf[:, kt, :], y_ps[:])
                else:
                    nc.scalar.copy(ybf[:, kt, :], y_ps[:])
            T["ybf"] = ybf
            if T["dump"]:
                nc.sync.dma_start(dbg["ybf"], ybf[:])
                nc.sync.dma_start(dbg["qn"], qn[:])
            ysq = work.tile([PT, NKT, ST], BF16, tag="ysq")
            nc.vector.tensor_tensor(ysq[:], ybf[:], ybf[:], op=OP.mult)
            # ---- LN2 stats via ones-matmuls ----
            ssum_ps = ps_misc.tile([1, ST], F32, tag="misc")
            for kt in range(NKT):
                nc.tensor.matmul(ssum_ps[:], ones128[:], ybf[:, kt, :],
                                 start=(kt == 0), stop=(kt == NKT - 1))
            ssq_ps = ps_misc.tile([1, ST], F32, tag="misc")
            for kt in range(NKT):
                nc.tensor.matmul(ssq_ps[:], ones128[:], ysq[:, kt, :],
                                 start=(kt == 0), stop=(kt == NKT - 1))
            srow_a = stat.tile([1, ST], F32, tag="srow_a")
            nc.scalar.copy(srow_a[:], ssum_ps[:])
            srow_b = stat.tile([1, ST], F32, tag="srow_b")
            nc.scalar.copy(srow_b[:], ssq_ps[:])
            # transpose rows -> TM for cheap stat math
            st_ps = ps_misc.tile([PT, NTT, 2], F32, tag="misc")
            for t in range(NTT):
                nc.tensor.transpose(st_ps[:, t, 0:1],
                                    srow_a[:, t * PT:(t + 1) * PT],
                                    ident32[:1, :1])
                nc.tensor.transpose(st_ps[:, t, 1:2],
                                    srow_b[:, t * PT:(t + 1) * PT],
                                    ident32[:1, :1])
            stm = stat.tile([PT, NTT, 2], F32, tag="stm")
            nc.scalar.copy(stm[:], st_ps[:])
            mu2 = stat.tile([PT, NTT], F32, tag="mu2")
            nc.vector.tensor_scalar(mu2[:], stm[:, :, 0], 1.0 / D, None,
                                    op0=OP.mult)
            v2 = stat.tile([PT, NTT], F32, tag="v2")
            nc.vector.tensor_scalar(v2[:], stm[:, :, 1], 1.0 / D, 1e-5,
                                    op0=OP.mult, op1=OP.add)
            v2b = stat.tile([PT, NTT], F32, tag="v2b")
            nc.vector.scalar_tensor_tensor(v2b[:], mu2[:], -1.0, mu2[:],
                                           op0=OP.mult, op1=OP.mult)
            nc.vector.tensor_tensor(v2[:], v2[:], v2b[:], op=OP.add)
            nc.vector.tensor_scalar(v2[:], v2[:], 1e-6, None, op0=OP.max)
            rstd2 = _quake_rsqrt(nc, stat, [PT, NTT], v2, "m2q", iters=1)
            ab_tm = stat.tile([PT, NTT, 2], BF16, tag="ab_tm")
            nc.vector.tensor_copy(ab_tm[:, :, 0], rstd2[:])
            nc.vector.scalar_tensor_tensor(ab_tm[:, :, 1], mu2[:], -1.0,
                                           rstd2[:], op0=OP.mult, op1=OP.mult)
            ab_ps = ps_misc.tile([1, 2 * ST], BF16, tag="misc")
            for t in range(NTT):
                nc.tensor.transpose(ab_ps[:, t * PT:(t + 1) * PT],
                                    ab_tm[:, t, 0:1], ident[:])
                nc.tensor.transpose(ab_ps[:, ST + t * PT:ST + (t + 1) * PT],
                                    ab_tm[:, t, 1:2], ident[:])
            ab_row = io.tile([1, 2 * ST], BF16, tag="ab_row")
            nc.scalar.copy(ab_row[:], ab_ps[:])
            T["ab_row"] = ab_row

        def back_a(S):
            b, dump = S["b"], S["dump"]
            ab_row, ybf = S["ab_row"], S["ybf"]
            if dump:
                nc.sync.dma_start(dbg["abrow"], ab_row[:])
            ab_bc = stat.tile([PT, 2, ST], BF16, tag="ab_bc")
            for i in range(2):
                abb_ps = ps_mm.tile([PT, ST], F32, tag="mm")
                nc.tensor.matmul(abb_ps[:], ones1r[:],
                                 ab_row[:, i * ST:(i + 1) * ST],
                                 start=True, stop=True)
                nc.vector.tensor_copy(ab_bc[:, i, :], abb_ps[:])
            # ---- z, stylize, silu(tanh) ----
            u1 = work.tile([PT, NKT, ST], BF16, tag="u1")
            aBb = ab_bc[:, 0, :].unsqueeze(1).broadcast_to([PT, NKT, ST])
            bBb = ab_bc[:, 1, :].unsqueeze(1).broadcast_to([PT, NKT, ST])
            nc.vector.tensor_tensor(u1[:], ybf[:], aBb, op=OP.mult)
            u2 = work.tile([PT, NKT, ST], BF16, tag="u2")
            nc.vector.tensor_tensor(u2[:], u1[:], bBb, op=OP.add)
            t1 = work.tile([PT, NKT, ST], BF16, tag="t1")
            for kt in range(NKT):
                nc.vector.tensor_scalar(
                    t1[:, kt, :], u2[:, kt, :], effSH[:, kt, b:b + 1],
                    effSH[:, kt, B_loc + b:B_loc + b + 1],
                    op0=OP.mult, op1=OP.add)
            th = work.tile([PT, NKT, ST], BF16, tag="th")
            nc.scalar.activation(th[:], t1[:], AF.Tanh, bias=0.0, scale=0.5)
            S["t1"], S["th"] = t1, th

        def back_b(S):
            b, st, dump = S["b"], S["st"], S["dump"]
            x_in, t1, th = S["x_in"], S["t1"], S["th"]
            tok0 = st * ST
            hs = work.tile([PT, NKT, ST], F8, tag="hs")
            nc.vector.scalar_tensor_tensor(hs[:], th[:], 1.0, t1[:],
                                           op0=OP.add, op1=OP.mult)
            if dump:
                nc.sync.dma_start(dbg["hs"], hs[:])
            # ---- out-proj (fp8 DoubleRow, TM out) + residual ----
            o_out = io.tile([PT, NTT, D], BF16, tag="o_out")
            for t in range(NTT):
                o_ps = ps_mm.tile([PT, ST], F32, tag="mm")
                for k2 in range(2):
                    nc.tensor.matmul(
                        o_ps[:],
                        hs[:, 2 * k2:2 * k2 + 2, t * PT:(t + 1) * PT],
                        ow[:, 2 * k2:2 * k2 + 2, :], start=(k2 == 0),
                        stop=(k2 == 1),
                        perf_mode=mybir.MatmulPerfMode.DoubleRow)
                nc.vector.scalar_tensor_tensor(
                    o_out[:, t, :], o_ps[:], 1.0 / W8SCALE, x_in[:, t, :],
                    op0=OP.mult, op1=OP.add)
            nc.scalar.dma_start(
                out_d[b, tok0:tok0 + ST, :].rearrange(
                    "(tt p) d -> p tt d", p=PT), o_out[:])

        phase_k(0)
        phase_k(1)
        pend = None
        for b in range(B_loc):
            for st in range(N_ST):
                dump = bool(b == 0 and st == 0 and dbg)
                T = dict(b=b, st=st, dump=dump)
                front_a(T)
                if pend is not None:
                    back_a(pend)
                front_b(T)
                if pend is not None:
                    back_b(pend)
                front_c(T)
                pend = T
                if st == 0 and b + 2 < B_loc:
                    phase_k(b + 2)
        back_a(pend)
        back_b(pend)


_PROG_CACHE = {}


def _get_program(B_loc):
    if B_loc not in _PROG_CACHE:
        _PROG_CACHE[B_loc] = build_program(B_loc)
    return _PROG_CACHE[B_loc]


def _prep_inputs(inputs):
    import ml_dtypes
    f32 = lambda a: np.asarray(a, dtype=np.float32)
    bf16 = lambda a: np.asarray(a).astype(np.float32).astype(ml_dtypes.bfloat16)
    fp8 = lambda a: np.asarray(a).astype(np.float32).astype(
        ml_dtypes.float8_e4m3fn)
    x = inputs["x"]; xf = inputs["xf"]; emb = inputs["emb"]
    ln_x_w = f32(inputs["ln_x_w"]); ln_x_b = f32(inputs["ln_x_b"])
    ln_t_w = f32(inputs["ln_t_w"]); ln_t_b = f32(inputs["ln_t_b"])
    Wq = f32(inputs["Wq"]); bq = f32(inputs["bq"])
    Wk = f32(inputs["Wk"]); bk = f32(inputs["bk"])
    Wv = f32(inputs["Wv"]); bv = f32(inputs["bv"])
    emb_w = f32(inputs["emb_w"]); emb_b = f32(inputs["emb_b"])
    ln_s_w = f32(inputs["ln_s_w"]); ln_s_b = f32(inputs["ln_s_b"])
    out_w = f32(inputs["out_w"]); out_b = f32(inputs["out_b"])
    gate = f32(inputs["gate"])
    assert not np.any(out_b), "out_b fold not implemented"
    assert not np.any(emb_b), "emb_b fold not implemented"

    # LN affine folded into projections
    wq_eff = Wq * ln_x_w[None, :]
    bq_eff = bq + Wq @ ln_x_b
    wk_eff = Wk * ln_t_w[None, :]
    bk_eff = bk + Wk @ ln_t_b
    wv_eff = Wv * ln_t_w[None, :]
    bv_eff = bv + Wv @ ln_t_b
    g = float(1.0 / (1.0 + np.exp(-gate[0])))

    mask8 = np.zeros((PT, NKT, H), np.float32)
    for p in range(PT):
        for kt in range(NKT):
            mask8[p, kt, 2 * kt + (1 if p >= 64 else 0)] = 1.0

    shared = {
        "wq": fp8(W8SCALE * wq_eff.T.copy()),
        "wk": bf16(wk_eff.T.copy()),
        "wv": bf16(wv_eff.T.copy()),
        # 0.5 from the tanh-silu identity, g from the output gate
        "ow": fp8(W8SCALE * 0.5 * g * out_w.T.copy()),
        "wemb": bf16(0.5 * emb_w.T.copy()),
        "bq": f32(bq_eff).reshape(NKT, PT).T.copy(),
        "bk": bf16(bk_eff)[None, :],
        "bv": bf16(bv_eff)[None, :],
        "lnsw": f32(ln_s_w)[None, :],
        "lnsb": f32(ln_s_b)[None, :],
        "mask8": bf16(mask8),
        "selq": bf16(mask8.transpose(2, 1, 0).copy()),
        "ident": bf16(np.eye(PT, dtype=np.float32)),
        "identg": bf16(W8SCALE * np.eye(PT, dtype=np.float32)),
        "ident32": np.eye(PT, dtype=np.float32),
    }
    in_maps = []
    xb = bf16(x); xfb = bf16(xf); embb = bf16(emb)
    for c in range(NCORES):
        m = dict(shared)
        m["x"] = xb[c * B_LOC:(c + 1) * B_LOC]
        m["xf"] = xfb[c * B_LOC:(c + 1) * B_LOC]
        m["emb"] = embb[c * B_LOC:(c + 1) * B_LOC]
        in_maps.append(m)
    return in_maps


def run(inputs, trace=False):
    in_maps = _prep_inputs(inputs)
    nc = _get_program(B_LOC)
    res = bass_utils.run_bass_kernel_spmd(
        nc, in_maps, core_ids=list(range(NCORES)), trace=trace)
    out = np.concatenate([np.asarray(r["out"]) for r in res.results], axis=0)
    return out.astype(np.float32), res


def kernel(**inputs):
    out, _ = run(inputs, trace=False)
    return out
